# revision 6
# baseline (speedup 1.0000x reference)
"""Trainium2 Bass kernel for a 3-layer GCN + mean-pool + MLP + softmax.

Reference computation (N=16384 nodes, dense adjacency):
    Ahat = D^-1/2 (A + I) D^-1/2
    H0 = X;  H_{l+1} = relu(Ahat @ (H_l @ W_l) + b_l)   l = 0,1,2
    g = mean(H3, axis=0);  h1 = elu(g @ Wh1 + bh1)
    logits = h1 @ Wh2 + bh2;  probs = softmax(logits)

Distribution (8 NeuronCores, 1D node/row parallel), v2 schedule:
  - Host folds the degree normalization into the adjacency and ships each
    core the transposed normalized adjacency columns for its 2048 output
    nodes as fp8 e4m3 (32MB/core), pre-tiled to [q, hb, g, p, t, i] so
    every 2MB group DMA reads 16KB per-partition-contiguous runs.
  - KEY CHANGE vs v1: output-chunk-outer schedule.  Each layer computes
    its 2048 output columns in TWO chunks of 1024, each chunk contracting
    over the full j=16384 stream (16MB of A).  Chunk 0 finalizes at 50%
    of the layer, chunk 1 at 100%.  The next layer's stationary Y is
    gathered in two node-halves: AllGather(h0) fires at 50% of the layer
    and is needed at 0% of the next; AllGather(h1) fires at 100% and is
    needed at 25% of the next.  Collectives therefore hide under the
    adjacency stream instead of sitting exposed at the layer boundary
    (v1 lost ~40us per boundary to exposed AllGather + rank skew).
  - Contraction order within a chunk is hb-major (all j with local node
    index < 1024 first), so stat[h1] is needed only 25% into the chunk
    stream.
  - Mean pool: per-chunk partial reduction overlaps the stream; partials
    are combined with an AllGather of 256B partials + local sum (4.6us
    floor) instead of an AllReduce (9.7us floor).
  - HAM keep-alive: tiny dummy matmuls after each group in the DMA-bound
    layer 0 keep the PE clock-gate at 2.4GHz through the per-group DMA
    slack (v1 oscillated between 1.2/2.4GHz, running many matmuls 2x
    slow).
  - 6 of 16 groups are cached in SBUF across layers (fills during layer
    0); 3 streaming buffers.  DMA ring split: adjacency on the SP
    (nc.sync) HWDGE ring; everything else on the ACT (nc.scalar) ring.
"""

import numpy as np
import ml_dtypes

N = 16384
NCORES = 8
ROWS = N // NCORES          # 2048 output nodes per core
P = 128
DIMS = [64, 32, 48, 64]     # feature dims: in, after l0, l1, l2
NQ = 2                      # output chunks per layer (1024 cols each)
QCH = ROWS // NQ            # 1024
NHB = 2                     # j-halves by local node index (stationary gather halves)
NG = 4                      # 2MB groups per (q, hb)
NGROUPS = NQ * NHB * NG     # 16 groups of 2MB per layer
NT = 16                     # 128-row j-stripes per group
NDT = NT // 2               # 8 DoubleRow double-stripes per group
ASCALE = 16.0               # fp8 range helper for Ahat
XSCALE = 16.0               # fp8 range helper for X
ABUFS = 3                   # streamed adjacency groups in flight (6MB)
NCACHE = 6                  # adjacency groups cached in SBUF across layers
CACHED_IDS = (0, 3, 6, 9, 12, 15)   # stream positions cached (of 16)

_nc_cache = None


def _build_nc():
    from concourse import bacc, mybir, tile

    dt = mybir.dt
    F32 = dt.float32
    F8 = dt.float8e4
    BF16 = dt.bfloat16
    AF = mybir.ActivationFunctionType
    OP = mybir.AluOpType
    DR = mybir.MatmulPerfMode.DoubleRow

    nc = bacc.Bacc(
        "TRN2", target_bir_lowering=False, debug=False, num_devices=NCORES
    )

    # adjacency pre-tiled on host: [q, hb, g, p, t, i]
    #   j = (2g + t//8)*2048 + hb*1024 + (t%8)*128 + p
    #   i_global = r_self*2048 + q*1024 + i
    a_t = nc.dram_tensor(
        "a_t", [NQ, NHB, NG, P, NT, QCH], F8, kind="ExternalInput"
    )
    # full X (scaled, fp8), stationary layout: [hb, p, r, u, c]
    #   node j = r*2048 + hb*1024 + u*128 + p
    x8 = nc.dram_tensor(
        "x8", [NHB, P, NCORES, 8, DIMS[0]], F8, kind="ExternalInput"
    )
    w_d = [
        nc.dram_tensor(
            f"w{l}", [DIMS[l], DIMS[l + 1]], F32 if l == 0 else BF16,
            kind="ExternalInput",
        )
        for l in range(3)
    ]
    b_d = [
        nc.dram_tensor(f"b{l}", [DIMS[l + 1], 1], F32, kind="ExternalInput")
        for l in range(3)
    ]
    wh1_d = nc.dram_tensor("wh1", [DIMS[3], 32], F32, kind="ExternalInput")
    bh1_d = nc.dram_tensor("bh1", [32, 1], F32, kind="ExternalInput")
    wh2_d = nc.dram_tensor("wh2", [32, 2], F32, kind="ExternalInput")
    bh2_d = nc.dram_tensor("bh2", [2, 1], F32, kind="ExternalInput")
    logits_o = nc.dram_tensor("logits", [2, 1], F32, kind="ExternalOutput")
    probs_o = nc.dram_tensor("probs", [2, 1], F32, kind="ExternalOutput")

    rg = [list(range(NCORES))]

    with tile.TileContext(nc) as tc:
        with (
            tc.tile_pool(name="const", bufs=1) as const,
            tc.tile_pool(name="apool", bufs=ABUFS) as apool,
            tc.tile_pool(name="cpool", bufs=NCACHE) as cpool,
            tc.tile_pool(name="spool", bufs=2) as spool,
            tc.tile_pool(name="hpool", bufs=2) as hpool,
            tc.tile_pool(name="ppool", bufs=2) as ppool,
            tc.tile_pool(name="ypool", bufs=2) as ypool,
            tc.tile_pool(name="smal", bufs=1) as smal,
            tc.tile_pool(name="accp", bufs=1, space="PSUM") as accp,
            tc.tile_pool(name="psml", bufs=3, space="PSUM") as psml,
            tc.tile_pool(name="psmlp", bufs=1, space="PSUM") as psmlp,
            tc.tile_pool(name="dram", bufs=1, space="DRAM") as dram,
        ):
            # ---- layer-0 stationary = X itself (fp8, host-tiled),
            #      loaded first so the tensor engine can start ASAP ----
            statX = [
                const.tile([P, NCORES, 8, DIMS[0]], F8, name=f"sx{hb}")
                for hb in range(NHB)
            ]
            for hb in range(NHB):
                nc.scalar.dma_start(statX[hb][:], x8.ap()[hb])

            # ---- PE pre-warm: dummy matmuls during the DMA ramp flip the
            #      HAM clock gate to 2.4 GHz before the real work arrives
            dm_w = smal.tile([P, 8], F8, name="dmw")
            nc.vector.memset(dm_w[:], 0.0)
            dm_x = smal.tile([P, 512], F8, name="dmx")
            nc.vector.memset(dm_x[:], 0.0)
            for i in range(40):
                dps = psmlp.tile([8, 512], F32, tag="dum", name=f"dps{i}")
                nc.tensor.matmul(
                    dps[:], lhsT=dm_w[:], rhs=dm_x[:], start=True, stop=True
                )

            def keepalive(idx):
                dps = psmlp.tile([8, 64], F32, tag="dum", name=f"ka{idx}")
                nc.tensor.matmul(
                    dps[:], lhsT=dm_w[:], rhs=dm_x[:, 0:64],
                    start=True, stop=True,
                )

            # ---- constants into SBUF (ACT ring — keep SP ring for A) ----
            def load(handle, shape, name, dtype=F32):
                t = const.tile(shape, dtype, name=name)
                nc.scalar.dma_start(t[:], handle.ap())
                return t

            w_sb = [
                load(
                    w_d[l], [DIMS[l], DIMS[l + 1]], f"w{l}sb",
                    dtype=F32 if l == 0 else BF16,
                )
                for l in range(3)
            ]
            b_sb = [load(b_d[l], [DIMS[l + 1], 1], f"b{l}sb") for l in range(3)]
            wh1_sb = load(wh1_d, [DIMS[3], 32], "wh1sb")
            bh1_sb = load(bh1_d, [32, 1], "bh1sb")
            wh2_sb = load(wh2_d, [32, 2], "wh2sb")
            bh2_sb = load(bh2_d, [2, 1], "bh2sb")

            a_cached = {}
            stat = None           # gathered stationary for current layer (l>=1)
            gpart = []            # mean-pool partials (layer 2)
            for l in range(3):
                c_stat = DIMS[0] if l == 0 else DIMS[l + 1]
                c_out = DIMS[l + 1]
                c_next = DIMS[l + 2] if l < 2 else None
                h_sb = hpool.tile([c_out, ROWS], BF16, tag="h", name=f"h{l}")
                stat_next = (
                    [
                        spool.tile(
                            [P, NCORES, 8, c_next], F8,
                            tag=f"st{hb}", name=f"st{l + 1}_{hb}",
                        )
                        for hb in range(NHB)
                    ]
                    if l < 2
                    else None
                )
                gi = 0
                ag_outs = []
                for q in range(NQ):
                    acc = accp.tile(
                        [c_stat, ROWS // NQ], F32, tag=f"acc{q}", name=f"acc{l}_{q}"
                    )
                    for hb in range(NHB):
                        for g in range(NG):
                            if gi in CACHED_IDS:
                                if l == 0:
                                    a_sb = cpool.tile(
                                        [P, NT, QCH], F8, tag="ac", name=f"ac{gi}"
                                    )
                                    nc.sync.dma_start(a_sb[:], a_t.ap()[q, hb, g])
                                    a_cached[gi] = a_sb
                                else:
                                    a_sb = a_cached[gi]
                            else:
                                a_sb = apool.tile(
                                    [P, NT, QCH], F8, tag="a",
                                    name=f"a{l}_{gi}",
                                )
                                nc.sync.dma_start(a_sb[:], a_t.ap()[q, hb, g])
                            first = hb == 0 and g == 0
                            last = hb == NHB - 1 and g == NG - 1
                            for t2 in range(NDT):
                                r = 2 * g + t2 // 4
                                u2 = t2 % 4
                                if l == 0:
                                    lw = statX[hb][:, r, 2 * u2 : 2 * u2 + 2, :]
                                else:
                                    lw = stat[hb][:, r, 2 * u2 : 2 * u2 + 2, :]
                                for ih in range(2):
                                    nc.tensor.matmul(
                                        acc[:, ih * 512 : (ih + 1) * 512],
                                        lhsT=lw,
                                        rhs=a_sb[
                                            :, 2 * t2 : 2 * t2 + 2,
                                            ih * 512 : (ih + 1) * 512,
                                        ],
                                        start=(first and t2 == 0),
                                        stop=(last and t2 == NDT - 1),
                                        perf_mode=DR,
                                    )
                            if l == 0:
                                keepalive(gi)
                            gi += 1

                    # ---- chunk q finalized: H columns [q*1024, (q+1)*1024) ----
                    if l == 0:
                        # H1 chunk = relu((Ahat@X)chunk @ W0 / s + b0)
                        p1 = ppool.tile([DIMS[0], QCH], F32, tag="p1", name=f"p1_{q}")
                        nc.vector.tensor_copy(out=p1[:], in_=acc[:])
                        for ih in range(2):
                            ps2 = psml.tile(
                                [DIMS[1], 512], F32, tag="psy", name=f"ps2_{q}_{ih}"
                            )
                            nc.tensor.matmul(
                                ps2[:], lhsT=w_sb[0][:],
                                rhs=p1[:, ih * 512 : (ih + 1) * 512],
                                start=True, stop=True,
                            )
                            nc.scalar.activation(
                                h_sb[:, q * QCH + ih * 512 : q * QCH + (ih + 1) * 512],
                                ps2[:],
                                AF.Relu,
                                bias=b_sb[0][:],
                                scale=1.0 / (ASCALE * XSCALE),
                            )
                    else:
                        for ih in range(2):
                            nc.scalar.activation(
                                h_sb[:, q * QCH + ih * 512 : q * QCH + (ih + 1) * 512],
                                acc[:c_out, ih * 512 : (ih + 1) * 512],
                                AF.Relu,
                                bias=b_sb[l][:],
                                scale=1.0 / ASCALE,
                            )

                    if l < 2:
                        # ---- project Y_{l+1} rows for this chunk's nodes and
                        #      AllGather them as the next layer's stationary ----
                        y_sb = ypool.tile(
                            [P, 8, c_next], F8, tag="y", name=f"y{l}_{q}"
                        )
                        for u in range(8):
                            n0 = q * QCH + u * P
                            ps = psml.tile(
                                [P, c_next], F32, tag="psy", name=f"psy{l}_{q}_{u}"
                            )
                            nc.tensor.matmul(
                                ps[:],
                                lhsT=h_sb[:, n0 : n0 + P],
                                rhs=w_sb[l + 1][:],
                                start=True,
                                stop=True,
                            )
                            nc.vector.tensor_copy(out=y_sb[:, u, :], in_=ps[:])
                        ag_in = dram.tile([P, 8, c_next], F8, name=f"agin{l}_{q}")
                        ag_out = dram.tile(
                            [NCORES, P, 8, c_next], F8, name=f"agout{l}_{q}",
                            addr_space="Shared",
                        )
                        nc.gpsimd.dma_start(ag_in[:], y_sb[:])
                        nc.gpsimd.collective_compute(
                            "AllGather",
                            OP.bypass,
                            replica_groups=rg,
                            ins=[ag_in[:].opt()],
                            outs=[ag_out[:].opt()],
                        )
                        ag_outs.append(ag_out)
                    else:
                        # ---- mean-pool partial for this chunk (overlaps stream)
                        gq = smal.tile([DIMS[3], 1], F32, name=f"gp{q}")
                        nc.vector.tensor_reduce(
                            gq[:], h_sb[:, q * QCH : (q + 1) * QCH],
                            axis=mybir.AxisListType.X, op=OP.add,
                        )
                        gpart.append(gq)
                # stat loads LAST on the ACT ring: they are the only
                # instructions that wait on AllGather completion; putting
                # them after both chunks' relu+agin keeps those posting
                # promptly (no head-of-line blocking on the ACT FIFO).
                for q, ag_out in enumerate(ag_outs):
                    nc.gpsimd.dma_start(
                        stat_next[q][:],
                        ag_out[:].rearrange("r p u c -> p r u c"),
                    )
                stat = stat_next

            # ---- combine partials; AllGather 256B partials + local sum ----
            gp = smal.tile([DIMS[3], 1], F32, name="gpart")
            nc.vector.tensor_tensor(gp[:], gpart[0][:], gpart[1][:], OP.add)
            ar_in = dram.tile([DIMS[3], 1], F32, name="arin")
            ar_out = dram.tile(
                [NCORES, DIMS[3], 1], F32, name="arout", addr_space="Shared"
            )
            nc.gpsimd.dma_start(ar_in[:], gp[:])
            nc.gpsimd.collective_compute(
                "AllGather",
                OP.bypass,
                replica_groups=rg,
                ins=[ar_in[:].opt()],
                outs=[ar_out[:].opt()],
            )
            g_all = smal.tile([DIMS[3], NCORES], F32, name="gall")
            nc.gpsimd.dma_start(g_all[:], ar_out[:].rearrange("r c o -> c (r o)"))
            g_sb = smal.tile([DIMS[3], 1], F32, name="gsb")
            nc.vector.tensor_reduce(
                g_sb[:], g_all[:], axis=mybir.AxisListType.X, op=OP.add
            )
            nc.any.tensor_scalar_mul(g_sb[:], g_sb[:], 1.0 / N)

            # ---- MLP head: h1 = elu(g @ Wh1 + bh1) ----
            ps1 = psmlp.tile([32, 1], F32, tag="dum", name="ps1")
            nc.tensor.matmul(ps1[:], lhsT=wh1_sb[:], rhs=g_sb[:], start=True, stop=True)
            # elu(x) = relu(x) + exp(min(x, 0)) - 1
            tmin = smal.tile([32, 1], F32, name="tmin")
            nc.vector.tensor_scalar(tmin[:], ps1[:], bh1_sb[:], 0.0, OP.add, OP.min)
            e1 = smal.tile([32, 1], F32, name="e1")
            nc.scalar.activation(e1[:], tmin[:], AF.Exp)
            r1 = smal.tile([32, 1], F32, name="r1")
            nc.scalar.activation(r1[:], ps1[:], AF.Relu, bias=bh1_sb[:])
            h1 = smal.tile([32, 1], F32, name="h1")
            nc.vector.tensor_tensor(h1[:], e1[:], r1[:], OP.add)
            nc.vector.tensor_scalar_add(h1[:], h1[:], -1.0)

            # ---- logits = h1 @ Wh2 + bh2; probs = softmax(logits) ----
            ps2m = psmlp.tile([2, 1], F32, tag="dum", name="ps2m")
            nc.tensor.matmul(ps2m[:], lhsT=wh2_sb[:], rhs=h1[:], start=True, stop=True)
            logit_sb = smal.tile([2, 1], F32, name="logitsb")
            nc.vector.tensor_scalar(logit_sb[:], ps2m[:], bh2_sb[:], None, OP.add)
            nc.scalar.dma_start(logits_o.ap(), logit_sb[:])

            e2 = smal.tile([2, 1], F32, name="e2")
            nc.scalar.activation(e2[:], ps2m[:], AF.Exp, bias=bh2_sb[:])
            ones21 = smal.tile([2, 1], F32, name="ones21")
            nc.any.memset(ones21[:], 1.0)
            ones12 = smal.tile([1, 2], F32, name="ones12")
            nc.any.memset(ones12[:], 1.0)
            ps3 = psmlp.tile([1, 1], F32, tag="dum", name="ps3")
            nc.tensor.matmul(ps3[:], lhsT=e2[:], rhs=ones21[:], start=True, stop=True)
            rsc = smal.tile([1, 1], F32, name="rsc")
            nc.vector.reciprocal(rsc[:], ps3[:])
            ps4 = psmlp.tile([2, 1], F32, tag="dum", name="ps4")
            nc.tensor.matmul(ps4[:], lhsT=ones12[:], rhs=rsc[:], start=True, stop=True)
            probs_sb = smal.tile([2, 1], F32, name="probssb")
            nc.vector.tensor_tensor(probs_sb[:], e2[:], ps4[:], OP.mult)
            nc.scalar.dma_start(probs_o.ap(), probs_sb[:])

    nc.finalize()
    return nc


def _install_ntff_hook():
    """Register the axon NTFF profiling hook if the container's antenv stub
    lacks it (bass_utils imports antenv.axon_hooks when trace=True)."""
    import sys
    import types

    try:
        import antenv.axon_hooks  # noqa: F401
        return
    except ImportError:
        pass
    mod = types.ModuleType("antenv.axon_hooks")
    _h = [None]
    mod.set_axon_ntff_profile_hook = lambda h: _h.__setitem__(0, h)
    mod.get_axon_ntff_profile_hook = lambda: _h[0]
    sys.modules["antenv.axon_hooks"] = mod
    import antenv

    antenv.axon_hooks = mod
    try:
        from trn_agent_boot import trn_boot

        hook = trn_boot._ntff_profile_via_ctypes("/opt/axon/libaxon_pjrt.so")
        if hook is not None:
            mod.set_axon_ntff_profile_hook(hook)
    except Exception:
        pass


def _get_nc():
    global _nc_cache
    if _nc_cache is None:
        _nc_cache = _build_nc()
    return _nc_cache


_last_results = None


def kernel(
    node_feat,
    adj_matrix,
    W0,
    b0,
    W1,
    b1,
    W2,
    b2,
    Wh1,
    bh1,
    Wh2,
    bh2,
):
    global _last_results
    import os

    node_feat = np.ascontiguousarray(np.asarray(node_feat, dtype=np.float32))
    adj = np.asarray(adj_matrix, dtype=np.float32)

    # ---- host-side sharding / preprocessing ----
    deg = adj.sum(axis=1, dtype=np.float32) + 1.0
    dinv = (1.0 / np.sqrt(deg)).astype(np.float32)

    fp8 = ml_dtypes.float8_e4m3
    bf16 = ml_dtypes.bfloat16
    f32c = lambda a, shape=None: np.ascontiguousarray(
        np.asarray(a, dtype=np.float32).reshape(shape)
        if shape is not None
        else np.asarray(a, dtype=np.float32)
    )

    # X scaled to fp8, stationary layout [hb, p, r, u, c]:
    #   node j = r*2048 + hb*1024 + u*128 + p
    x8 = (node_feat * np.float32(XSCALE)).astype(fp8)
    x8 = np.ascontiguousarray(
        x8.reshape(NCORES, NHB, 8, P, DIMS[0]).transpose(1, 3, 0, 2, 4)
    )

    common = {
        "x8": x8,
        "w0": f32c(W0),
        "b0": f32c(b0, (-1, 1)),
        "w1": np.ascontiguousarray(np.asarray(W1, np.float32)).astype(bf16),
        "b1": f32c(b1, (-1, 1)),
        "w2": np.ascontiguousarray(np.asarray(W2, np.float32)).astype(bf16),
        "b2": f32c(b2, (-1, 1)),
        "wh1": f32c(Wh1),
        "bh1": f32c(bh1, (-1, 1)),
        "wh2": f32c(Wh2),
        "bh2": f32c(bh2, (-1, 1)),
    }

    in_maps = []
    idx = np.arange(ROWS)
    sdinv = dinv * np.float32(ASCALE)
    for k in range(NCORES):
        sl = slice(k * ROWS, (k + 1) * ROWS)
        # rows of ASCALE*Ahat for this core's output nodes
        blk = adj[sl, :] * sdinv[sl, None]
        blk *= dinv[None, :]
        blk[idx, k * ROWS + idx] = sdinv[sl] * dinv[sl]  # + I self loops
        a_k = blk.T.astype(fp8)  # [N, ROWS] = scaled Ahat.T cols
        # pre-tile to device layout [q, hb, g, p, t, i]:
        #   j = (2g + t//8)*2048 + hb*1024 + (t%8)*128 + p
        #   i_global(col of blk) = q*1024 + i
        a_k = a_k.reshape(NG, 2, NHB, 8, P, NQ, QCH)  # g, r2, hb, ts, p, q, i
        a_k = np.ascontiguousarray(
            a_k.transpose(5, 2, 0, 4, 1, 3, 6).reshape(NQ, NHB, NG, P, NT, QCH)
        )
        m = {"a_t": a_k}
        m.update(common)
        in_maps.append(m)

    from concourse import bass_utils

    nc = _get_nc()
    trace = bool(int(os.environ.get("GCN_TRACE", "0")))
    if trace:
        _install_ntff_hook()
    res = bass_utils.run_bass_kernel_spmd(
        nc, in_maps, core_ids=list(range(NCORES)), trace=trace
    )
    _last_results = res

    out0 = res.results[0]
    logits = np.asarray(out0["logits"], dtype=np.float32).reshape(2)
    probs = np.asarray(out0["probs"], dtype=np.float32).reshape(2)
    return (logits, probs)
